# revision 38
# baseline (speedup 1.0000x reference)
"""Masked-softmax cross-entropy loss on 8 Trainium2 cores — PE-matmul design.

Math per target row t (16384 rows, 4096 src cols):
  numer[t] = sum_j exp(x[t,j]/tau) over valid src cols j with color == tgt color t
  denom[t] = sum_j exp(x[t,j]/tau) over valid src cols j
  p_gt = numer/denom, nll = -log(p_gt + eps); rows with numer == 0 masked out.

Device formulation (per core: one batch-half, 2048 target rows):
  x is uploaded TRANSPOSED [4096_j, 2048_t] and cast to bf16 on the host
  (halves HBM traffic; validated end-to-end rel err ~1e-5 vs the 2e-2 gate),
  pre-tiled so every piece is a contiguous per-partition HWDGE load.
  ACT: exp(10*x^T) -> bf16 IN PLACE (tiles are write-once: load + own exp,
       then only read — so no WAR/WAW sems exist anywhere in the pipeline).
  PE : bucket sums S[c, t] = sum_j onehot[j, c] * et[j, t] via per-j-tile
       matmuls with a host-built one-hot color matrix as stationary weights
       (col c<98: sid_j == c; col 98: valid mask), accumulated in one
       [99, 2048] PSUM region (4 banks).
  Tail: one ACT copy PSUM->SBUF (bf16) + one small SWDGE store.  DVE is
  never used.  Host gathers numer[t] = S[tid_t, t], denom[t] = S[98, t]
  and runs the tiny 16K-scalar finalize.

Engine budget per core: ACT ~59us (the wall: exp at 1 elem/lane/cycle at
1.2GHz; note the core clock itself varies ~20% run-to-run), sync-ring DMA
16MB at ~300GB/s ~55us (fully overlapped), PE ~30-60us (p-state, hidden).

Schedule: ALL x rides the sync HWDGE ring — measured: SWDGE is ~100-150GB/s
and steals SDMA slots, and scalar-ring (ACT HWDGE) dispatches block the ACT
queue and slow activations ~20%, so both "dual ring" splits are net losses.
Piece sizes go fine->coarse->fine: eighth-chunks at the head so exp0 starts
~9us earlier and the DMA cold-start deficit drains without stalling ACT,
full chunks mid-stream to minimize ACT per-instruction overhead (~285ns
each), quarters at the tail so the final matmuls chase piece-sized exps.

Sync-wait budget (walrus: 1 sem wait per instruction, and Tile sem-ifies
even same-queue WAW on recycled slots): every load carries at most one
DMA-lane WAW wait, every exp exactly its own load's lane RAW, matmuls only
their exp's ACT wait (a PE warm-up ldweights absorbs the one-hot load wait),
the tail copy waits the last matmul, the store waits the copy.
"""

import os
import numpy as np

B = 4
S_TGT = 8
L_TGT = 512
C = 4
N = 4096          # src columns
P = 128
ROWS = 2048       # tgt rows per core (half a batch)
NCORES = 8
PAD = -1.0
EPS = 1e-15

KC = 99           # one-hot columns: 98 color ids + 1 valid-mask column
NJT = N // P      # 32 j-tiles
JPC = 4           # j-tiles per chunk
NCHUNK = NJT // JPC   # 8 chunks of [128, 8192]
CW = JPC * ROWS   # chunk free width = 8192

MBLK = 512        # matmul moving block (one PSUM bank of f32)

_NC_CACHE = {}


def _patch_split_drain():
    """Split the kernel-tail drain's sem waits across several drain
    instructions (walrus rejects >1 sync wait on one CTRL instruction)."""
    import concourse.tile as tile
    from concourse.vector_clock import ScopedClock, VectorClock

    if getattr(tile.TileContext, "_split_drain_patched", False):
        return

    def _drain_and_barrier(self, tick_clock, wait_clock):
        g = tick_clock.global_clock
        n = len(g)
        for base in range(n):
            vec = [g[i] if i == base else 0 for i in range(n)]
            if not any(vec):
                continue
            d = self.nc.sync.drain()
            wait_clock.add_sem_waits(d.ins, ScopedClock({None: VectorClock(vec)}))
        self.nc.all_engine_barrier()
        popped = self.nc._tile_sem_poison_stack.pop()
        assert popped is self._sem_poison
        self.nc.clear_and_free_semaphores(list(self.sems.allocated().values()))
        self.nc.all_engine_barrier()

    tile.TileContext._drain_and_barrier = _drain_and_barrier
    tile.TileContext._split_drain_patched = True


def _build_nc():
    import concourse.bass as bass
    import concourse.mybir as mybir
    import concourse.tile as tile
    from contextlib import ExitStack

    _patch_split_drain()
    nc = bass.Bass()
    f32 = mybir.dt.float32
    bf16 = mybir.dt.bfloat16

    # x^T pre-tiled: row ci*128+p holds [jt-local layout] = 4 j-tiles side
    # by side, 16KB contiguous per partition per chunk.  bf16 (validated
    # end-to-end rel err ~9e-6) so the exp can run IN PLACE — no second
    # tile pool, no slot recycling, every instruction carries <=1 sem wait.
    x = nc.declare_dram_parameter("x", [NCHUNK * P, CW], bf16, isOutput=False)
    oneh_d = nc.declare_dram_parameter("oneh", [P, NJT * KC], bf16,
                                       isOutput=False)
    s_out = nc.declare_dram_parameter("s", [KC, ROWS], bf16, isOutput=True)

    with tile.TileContext(nc) as tc:
        with ExitStack() as ctx:
            const_pool = ctx.enter_context(tc.tile_pool(name="const", bufs=1))
            # bufs=1 + a unique tag per tile: every x piece is its own
            # write-once allocation (a pool ring would multiply slots per
            # call-site tag and overflow SBUF).
            x_pool = ctx.enter_context(tc.tile_pool(name="x", bufs=1))
            res_pool = ctx.enter_context(tc.tile_pool(name="res", bufs=1))
            psum_pool = ctx.enter_context(
                tc.tile_pool(name="psum", bufs=1, space="PSUM")
            )

            oneh = const_pool.tile([P, NJT * KC], bf16)
            spsum = psum_pool.tile([P, ROWS], f32)
            ssb = res_pool.tile([P, ROWS], bf16)

            def xtile(w, tag):
                return x_pool.tile([P, w], bf16, name=tag, tag=tag)

            # ---- schedule: ALL x rides the sync HWDGE ring (measured
            # ~300GB/s; the SWDGE ring runs ~100-150GB/s and steals SDMA
            # slots, and the scalar HWDGE ring blocks the ACT queue and
            # slows activations ~20% — both splits measured SLOWER).  The
            # ring slightly outruns ACT (1.7us vs 2.0us per 512KB), so
            # piece sizes go fine->coarse: small pieces at the head so exp0
            # starts early and the cold-start deficit drains, full chunks
            # mid-stream for minimal ACT instruction overhead, quarters at
            # the tail so the last matmuls chase piece-sized exps.
            # Entries: (ci, col0, width).
            Q = ROWS
            sched = (
                [(0, 0, 1024), (0, 1024, 1024), (0, 2048, 1024),
                 (0, 3072, 1024), (0, 4096, Q), (0, 6144, Q)]
                + [(1, 0, CW // 2), (1, CW // 2, CW // 2)]
                + [(2, 0, CW // 2), (2, CW // 2, CW // 2)]
                + [(3, 0, CW // 2), (3, CW // 2, CW // 2)]
                + [(ci, 0, CW) for ci in (4, 5, 6)]
                + [(7, 0, Q), (7, 2048, Q), (7, 4096, Q),
                   (7, 6144, 1024), (7, 7168, 1024)]
            )

            # loads up front in consumption order (write-once tiles: no
            # WAR/WAW sems; HWDGE lane reuse puts at most one lane-WAW wait
            # on a load, its only wait).
            tiles = {}
            for ci, col0, w in sched:
                t = xtile(w, f"x{ci}_{col0}")
                nc.sync.dma_start(t[:], x[ci * P:(ci + 1) * P, col0:col0 + w])
                tiles[(ci, col0)] = t
            # one-hot matrix on the (otherwise idle) SWDGE ring; the PE
            # warm-up ldweights absorbs its DMA-lane wait so every real
            # matmul carries only its ACT sem wait.
            nc.gpsimd.dma_start(oneh[:], oneh_d[:])
            nc.tensor.ldweights(oneh[:, 0:KC])

            # PSUM accumulation flags: start on the first matmul EMITTED
            # per m-block, stop on the last.
            nmm, seen = {}, {}
            for ci, col0, w in sched:
                for k0 in range(0, w, MBLK):
                    mb = ((col0 + k0) % ROWS) // MBLK
                    nmm[mb] = nmm.get(mb, 0) + 1

            # ---- compute: exp in place (each tile written only by its own
            # load + exp, then read by matmuls), bucket matmuls per j-tile.
            for ci, col0, w in sched:
                t = tiles[(ci, col0)]
                nc.scalar.activation(
                    t[:], t[:], mybir.ActivationFunctionType.Exp, scale=10.0
                )
                for k0 in range(0, w, MBLK):
                    jt = ci * JPC + (col0 + k0) // ROWS
                    mb = ((col0 + k0) % ROWS) // MBLK
                    seen[mb] = seen.get(mb, 0) + 1
                    nc.tensor.matmul(
                        spsum[0:KC, mb * MBLK:(mb + 1) * MBLK],
                        oneh[:, jt * KC:(jt + 1) * KC],
                        t[:, k0:k0 + MBLK],
                        start=(seen[mb] == 1),
                        stop=(seen[mb] == nmm[mb]),
                    )

            # final PSUM->SBUF copy on ACT (free after the last exp; DVE is
            # then never used at all), bf16-cast, then one small SWDGE store.
            nc.scalar.copy(ssb[0:KC, :], spsum[0:KC, :])
            nc.gpsimd.dma_start(s_out[:], ssb[0:KC, :])
    return nc


def _get_nc():
    if "nc" not in _NC_CACHE:
        _NC_CACHE["nc"] = _build_nc()
    return _NC_CACHE["nc"]


def _color_ids(src, tgt):
    """Map each color row to a per-batch integer id via exact byte equality."""
    src_f = np.ascontiguousarray(src.reshape(B, -1, C))
    tgt_f = np.ascontiguousarray(tgt.reshape(B, -1, C))
    n_s = src_f.shape[1]
    src_ids = np.empty((B, n_s), np.int64)
    tgt_ids = np.empty((B, tgt_f.shape[1]), np.int64)
    for b in range(B):
        allc = np.ascontiguousarray(np.concatenate([src_f[b], tgt_f[b]], axis=0))
        view = allc.view([("", allc.dtype)] * C).reshape(-1)
        _, inv = np.unique(view, return_inverse=True)
        s_ids, t_ids = inv[:n_s].copy(), inv[n_s:].copy()
        s_ids[np.all(src_f[b] == PAD, axis=-1)] = -1
        t_ids[np.all(tgt_f[b] == PAD, axis=-1)] = -2
        src_ids[b], tgt_ids[b] = s_ids, t_ids
    return src_ids, tgt_ids


def kernel(seg_sim_map, seg_colors_src, seg_colors_tgt):
    import ml_dtypes
    from concourse.bass_utils import run_bass_kernel_spmd

    bf16 = ml_dtypes.bfloat16
    seg_sim_map = np.asarray(seg_sim_map, dtype=np.float32)
    src_ids, tgt_ids = _color_ids(
        np.asarray(seg_colors_src, np.float32), np.asarray(seg_colors_tgt, np.float32)
    )
    assert src_ids.max() < KC - 1, "color id overflows one-hot width"

    # per-batch one-hot color matrix [N, KC]: col c<98 = (sid == c),
    # col 98 = valid mask; pad columns are all-zero -> excluded exactly.
    oneh_b = []
    for b in range(B):
        oh = np.zeros((N, KC), np.float32)
        valid = src_ids[b] >= 0
        oh[np.arange(N)[valid], src_ids[b][valid]] = 1.0
        oh[valid, KC - 1] = 1.0
        oneh_b.append(
            np.ascontiguousarray(
                oh.reshape(NJT, P, KC).transpose(1, 0, 2).reshape(P, NJT * KC)
            ).astype(bf16)
        )

    in_maps = []
    for c in range(NCORES):
        b, h = c // 2, c % 2
        rows = slice(h * ROWS, (h + 1) * ROWS)
        xT = seg_sim_map[b, rows, :].T.astype(bf16)            # [N, ROWS]
        xh = np.ascontiguousarray(
            xT.reshape(NCHUNK, JPC, P, ROWS)
            .transpose(0, 2, 1, 3)
            .reshape(NCHUNK * P, CW)
        )
        in_maps.append({"x": xh, "oneh": oneh_b[b]})

    trace = os.environ.get("KERNEL_PROFILE", "") == "1"
    nc = _get_nc()
    out = run_bass_kernel_spmd(nc, in_maps, list(range(NCORES)), trace=trace)
    if trace and out.exec_time_ns is not None:
        print(f"HW exec time: {out.exec_time_ns} ns")
        print(f"HW exec mean: {out.mean_exec_time_ns} ns")

    numer = np.empty((B, N), np.float32)
    denom = np.empty((B, N), np.float32)
    for c in range(NCORES):
        b, h = c // 2, c % 2
        rows = slice(h * ROWS, (h + 1) * ROWS)
        r = out.results[c]
        S = np.asarray(r["s"], np.float32)
        tid = tgt_ids[b, rows]
        valid_t = tid >= 0
        nm = np.zeros(ROWS, np.float32)
        nm[valid_t] = S[tid[valid_t], np.arange(ROWS)[valid_t]]
        numer[b, rows] = nm
        denom[b, rows] = S[KC - 1, :]

    # host finalize, mirroring the reference ops in f32 (touches 16K scalars)
    p_gt = numer / denom
    nll = -np.log(p_gt + np.float32(EPS))
    m = (numer > 0).astype(np.float32)
    nll3 = nll.reshape(B, S_TGT, L_TGT)
    m3 = m.reshape(B, S_TGT, L_TGT)
    nvalid = m3.sum(-1)
    seg_loss = np.where(
        nvalid > 0, (nll3 * m3).sum(-1) / np.maximum(nvalid, np.float32(1.0)), 0.0
    ).astype(np.float32)
    cnt = int((nvalid > 0).sum())
    total = np.float32(seg_loss.sum(dtype=np.float32) / np.float32(max(cnt, 1)))
    return np.asarray(total, np.float32), np.asarray(cnt, np.int32)
